# revision 21
# baseline (speedup 1.0000x reference)
"""HCLT probabilistic-circuit kernel for 8 Trainium2 NeuronCores.

Math: the reference collapses algebraically. With
  lp0 + lp1 summed in log space, exp'd, mixed by w_sum, then logsumexp'd,
the whole network is
  out[b] = log( sum_{k,m} w_sum[k] * W0[k,m,x0_b] * W1[k,m,x1_b] )
        = log( A[x0_b, x1_b] ),   A = sum_k w_k * W0[k].T @ W1[k]  (shape [C, C])

Distribution: shard the latent axis k (256) across 8 cores (32 each). Each core
reads its W shard quantized to fp8e4 (sqrt(w_sum) folded into both factors plus
power-of-two range scales) and accumulates the partial A_c via DoubleRow fp8
matmuls (256 contraction rows per instruction at 2x rate). The host sums the 8
partial A_c [256, 256] f32 outputs, gathers at the 1024 (x0_b, x1_b) positions,
removes the scales, and takes the log.

Layout: both W factors live in ONE dram tensor, interleaved per 256-row chunk
(x0-block 512B | x1-block 512B per partition row), so a single DMA trigger
(~0.8us each on the issuing engine) feeds both matmul operands. Pieces are
graduated (tiny first) so the PE starts as soon as possible, and alternate
between the two trigger engines/queues. The partial A is DMA'd straight out
of PSUM, split across both queues.
"""

import math
import sys

import numpy as np

sys.path.insert(0, "/opt/trn_rl_repo")

import ml_dtypes

B, V, M, C = 1024, 2, 256, 256
NCORES = 8
KSH = M // NCORES          # k per core = 32
KM = KSH * M               # flattened contraction rows per core = 8192
NC2 = KM // 256            # 32 DoubleRow chunks of 256 rows
CW = 1024                  # sbuf columns per chunk: [x0 512 | x1 512]
# pieces as (start_chunk, end_chunk, engine): engine 0 = sync/q1,
# 1 = scalar/q10.  Uniform 4-chunk pieces keep DMA packets at 4KB (packet
# width = piece row width; small packets tank throughput).  Each queue
# streams its own contiguous chunk range; MM emission alternates between
# the queues' piece streams (PSUM accumulation is order-independent, so
# chunk order is free).
# q1 (sync) starts ~1.5us earlier than q10 (scalar), so it carries more.
PIECES = [
    (0, 4, 0),     # q1  ~10.3
    (18, 22, 1),   # q10 ~11.3 (after 128-row warmup clears its desc init)
    (4, 8, 0),     # q1  ~12.4
    (22, 26, 1),   # q10 ~13.4
    (8, 12, 0),    # q1  ~14.5
    (26, 30, 1),   # q10 ~15.4
    (12, 16, 0),   # q1  ~16.6
    (30, 32, 1),   # q10 ~16.5
    (16, 18, 0),   # q1  ~17.6
]

_cache = {}


def _build_program():
    import concourse.bacc as bacc
    import concourse.mybir as mybir
    from concourse.tile import TileContext

    bf16 = mybir.dt.bfloat16
    f32 = mybir.dt.float32
    fp8 = mybir.dt.float8e4

    nc = bacc.Bacc("TRN2", target_bir_lowering=False)

    # per-chunk free layout: x0: [h(2), i(2), m(128)] then x1: [i(2), n(256)]
    xw = nc.dram_tensor("xw", [128, NC2 * CW], fp8, kind="ExternalInput")
    warm = nc.dram_tensor("warm", [128, 64], fp8, kind="ExternalInput")
    aout0 = nc.dram_tensor("aout0", [128, C], bf16, kind="ExternalOutput")
    aout1 = nc.dram_tensor("aout1", [128, C], bf16, kind="ExternalOutput")

    with TileContext(nc) as tc:
        with (
            tc.tile_pool(name="wp", bufs=1) as wp,
            tc.tile_pool(name="apool", bufs=1, space="PSUM") as apool,
        ):
            xsb = wp.tile([128, NC2 * CW], fp8, name="xsb")
            warmsb1 = wp.tile([128, 64], fp8, name="warmsb1")

            # single-row DMA on the scalar queue first: pays that queue's
            # init cost off the critical path (q1 starts fast on its own)
            nc.scalar.dma_start(out=warmsb1[:], in_=warm[:])
            # graduated pieces; each engine's triggers issue in listed order
            for eng_id in (0, 1):
                eng = nc.sync if eng_id == 0 else nc.scalar
                for a, b, e in PIECES:
                    if e == eng_id:
                        sl = slice(a * CW, b * CW)
                        eng.dma_start(out=xsb[:, sl], in_=xw[:, sl])

            a_ps = []
            for h in range(2):
                ah = apool.tile([128, C], f32, name=f"a{h}")
                a_ps.append(ah)

            nmm = [0, 0]
            for a, b, _e in PIECES:
                for h in range(2):
                    for j in range(a, b):
                        lhsT = xsb[
                            :, j * CW + h * 256 : j * CW + (h + 1) * 256
                        ].rearrange("p (i m) -> p i m", i=2)
                        rhs = xsb[:, j * CW + 512 : (j + 1) * CW].rearrange(
                            "p (i n) -> p i n", i=2
                        )
                        nmm[h] += 1
                        nc.tensor.matmul(
                            a_ps[h],
                            lhsT=lhsT,
                            rhs=rhs,
                            start=(nmm[h] == 1),
                            stop=(nmm[h] == NC2),
                            perf_mode=mybir.MatmulPerfMode.DoubleRow,
                        )

            # PSUM -> SBUF (bf16) on two engines in parallel, then one
            # output DMA per queue
            asb0 = wp.tile([128, C], bf16, name="asb0")
            asb1 = wp.tile([128, C], bf16, name="asb1")
            nc.vector.tensor_copy(asb0, a_ps[0])
            nc.scalar.copy(asb1, a_ps[1])
            nc.sync.dma_start(out=aout0[:], in_=asb0[:])
            nc.scalar.dma_start(out=aout1[:], in_=asb1[:])

    nc.compile()
    return nc


def _prep_inputs(x, W, w_sum):
    fp8 = ml_dtypes.float8_e4m3
    x = np.asarray(x)
    W = np.asarray(W, dtype=np.float32)
    w_sum = np.asarray(w_sum, dtype=np.float32)

    sq = np.sqrt(w_sum).astype(np.float32)
    P0 = W[0] * sq[:, None, None]  # [M(k), M(m), C]
    P1 = W[1] * sq[:, None, None]
    S0 = 2.0 ** math.floor(math.log2(192.0 / float(P0.max())))
    S1 = 2.0 ** math.floor(math.log2(192.0 / float(P1.max())))
    Q0 = (P0 * np.float32(S0)).astype(fp8)
    Q1 = (P1 * np.float32(S1)).astype(fp8)

    warm = np.zeros((128, 64), dtype=fp8)
    in_maps = []
    for c in range(NCORES):
        k0 = c * KSH
        q0 = Q0[k0 : k0 + KSH].reshape(KM, C)
        q1 = Q1[k0 : k0 + KSH].reshape(KM, C)
        # x0 block: [p, j, h, i, m] = q0[j*256 + i*128 + p, h*128 + m]
        t0 = q0.reshape(NC2, 2, 128, 2, 128).transpose(2, 0, 3, 1, 4)
        t0 = t0.reshape(128, NC2, 512)
        # x1 block: [p, j, i, n] = q1[j*256 + i*128 + p, n]
        t1 = q1.reshape(NC2, 2, 128, C).transpose(2, 0, 1, 3)
        t1 = t1.reshape(128, NC2, 512)
        xwc = np.ascontiguousarray(
            np.concatenate([t0, t1], axis=2).reshape(128, NC2 * CW)
        )
        in_maps.append({"xw": xwc, "warm": warm})
    return in_maps, S0, S1


def _run(in_maps, **kwargs):
    from concourse.bass_utils import run_bass_kernel_spmd

    if "nc" not in _cache:
        _cache["nc"] = _build_program()
    return run_bass_kernel_spmd(
        _cache["nc"], in_maps, core_ids=list(range(NCORES)), **kwargs
    )


def _finish(res, x, S0, S1):
    x = np.asarray(x)
    asum = np.zeros((2, 128, C), dtype=np.float64)
    for r in res.results:
        asum[0] += r["aout0"].astype(np.float64)
        asum[1] += r["aout1"].astype(np.float64)
    # A[c0, c1] with c0 = h*128 + p
    A = asum.reshape(256, 256)
    vals = A[x[:, 0].astype(np.int64), x[:, 1].astype(np.int64)]
    return (np.log(vals) - math.log(S0 * S1)).astype(np.float32)


def kernel(x, W, w_sum):
    in_maps, S0, S1 = _prep_inputs(x, W, w_sum)
    res = _run(in_maps)
    return _finish(res, x, S0, S1)


# revision 22
# speedup vs baseline: 1.0629x; 1.0629x over previous
"""HCLT probabilistic-circuit kernel for 8 Trainium2 NeuronCores.

Math: the reference collapses algebraically. With
  lp0 + lp1 summed in log space, exp'd, mixed by w_sum, then logsumexp'd,
the whole network is
  out[b] = log( sum_{k,m} w_sum[k] * W0[k,m,x0_b] * W1[k,m,x1_b] )
        = log( A[x0_b, x1_b] ),   A = sum_k w_k * W0[k].T @ W1[k]  (shape [C, C])

Distribution: shard the latent axis k (256) across 8 cores (32 each). Each core
reads its W shard quantized to fp8e4 (sqrt(w_sum) folded into both factors plus
power-of-two range scales) and accumulates the partial A_c via DoubleRow fp8
matmuls (256 contraction rows per instruction at 2x rate). The host sums the 8
partial A_c [256, 256] f32 outputs, gathers at the 1024 (x0_b, x1_b) positions,
removes the scales, and takes the log.

Layout: both W factors live in ONE dram tensor, interleaved per 256-row chunk
(x0-block 512B | x1-block 512B per partition row), so a single DMA trigger
(~0.8us each on the issuing engine) feeds both matmul operands. Pieces are
graduated (tiny first) so the PE starts as soon as possible, and alternate
between the two trigger engines/queues. The partial A is DMA'd straight out
of PSUM, split across both queues.
"""

import math
import sys

import numpy as np

sys.path.insert(0, "/opt/trn_rl_repo")

import ml_dtypes

B, V, M, C = 1024, 2, 256, 256
NCORES = 8
KSH = M // NCORES          # k per core = 32
KM = KSH * M               # flattened contraction rows per core = 8192
NC2 = KM // 256            # 32 DoubleRow chunks of 256 rows
CW = 1024                  # sbuf columns per chunk: [x0 512 | x1 512]
# pieces as (start_chunk, end_chunk, engine): engine 0 = sync/q1,
# 1 = scalar/q10.  Uniform 4-chunk pieces keep DMA packets at 4KB (packet
# width = piece row width; small packets tank throughput).  Each queue
# streams its own contiguous chunk range; MM emission alternates between
# the queues' piece streams (PSUM accumulation is order-independent, so
# chunk order is free).
# q1 (sync) starts ~1.5us earlier than q10 (scalar), so it carries more.
# emission assumes worst-case q10 start (its init lag is ~randomly 1-3us);
# early work rides q1 only, q10 pieces are interleaved at their worst-case
# arrival positions (early q10 arrival costs nothing - data waits in SBUF).
PIECES = [
    (0, 4, 0),     # q1  ~10.3
    (4, 8, 0),     # q1  ~12.4
    (18, 22, 1),   # q10 ~13.2 worst-case
    (8, 12, 0),    # q1  ~14.5
    (22, 26, 1),   # q10 ~15.4
    (12, 16, 0),   # q1  ~16.6
    (26, 30, 1),   # q10 ~17.5
    (16, 18, 0),   # q1  ~17.6
    (30, 32, 1),   # q10 ~18.6
]

_cache = {}


def _build_program():
    import concourse.bacc as bacc
    import concourse.mybir as mybir
    from concourse.tile import TileContext

    bf16 = mybir.dt.bfloat16
    f32 = mybir.dt.float32
    fp8 = mybir.dt.float8e4

    nc = bacc.Bacc("TRN2", target_bir_lowering=False)

    # per-chunk free layout: x0: [h(2), i(2), m(128)] then x1: [i(2), n(256)]
    xw = nc.dram_tensor("xw", [128, NC2 * CW], fp8, kind="ExternalInput")
    aout0 = nc.dram_tensor("aout0", [128, C], bf16, kind="ExternalOutput")
    aout1 = nc.dram_tensor("aout1", [128, C], bf16, kind="ExternalOutput")

    with TileContext(nc) as tc:
        with (
            tc.tile_pool(name="wp", bufs=1) as wp,
            tc.tile_pool(name="apool", bufs=1, space="PSUM") as apool,
        ):
            xsb = wp.tile([128, NC2 * CW], fp8, name="xsb")

            # each engine's triggers issue in listed order
            for eng_id in (0, 1):
                eng = nc.sync if eng_id == 0 else nc.scalar
                for a, b, e in PIECES:
                    if e == eng_id:
                        sl = slice(a * CW, b * CW)
                        eng.dma_start(out=xsb[:, sl], in_=xw[:, sl])

            a_ps = []
            for h in range(2):
                ah = apool.tile([128, C], f32, name=f"a{h}")
                a_ps.append(ah)

            nmm = [0, 0]
            for a, b, _e in PIECES:
                for h in range(2):
                    for j in range(a, b):
                        lhsT = xsb[
                            :, j * CW + h * 256 : j * CW + (h + 1) * 256
                        ].rearrange("p (i m) -> p i m", i=2)
                        rhs = xsb[:, j * CW + 512 : (j + 1) * CW].rearrange(
                            "p (i n) -> p i n", i=2
                        )
                        nmm[h] += 1
                        nc.tensor.matmul(
                            a_ps[h],
                            lhsT=lhsT,
                            rhs=rhs,
                            start=(nmm[h] == 1),
                            stop=(nmm[h] == NC2),
                            perf_mode=mybir.MatmulPerfMode.DoubleRow,
                        )

            # PSUM -> SBUF (bf16) on two engines in parallel, then one
            # output DMA per queue
            asb0 = wp.tile([128, C], bf16, name="asb0")
            asb1 = wp.tile([128, C], bf16, name="asb1")
            nc.vector.tensor_copy(asb0, a_ps[0])
            nc.scalar.copy(asb1, a_ps[1])
            nc.sync.dma_start(out=aout0[:], in_=asb0[:])
            nc.scalar.dma_start(out=aout1[:], in_=asb1[:])

    nc.compile()
    return nc


def _prep_inputs(x, W, w_sum):
    fp8 = ml_dtypes.float8_e4m3
    x = np.asarray(x)
    W = np.asarray(W, dtype=np.float32)
    w_sum = np.asarray(w_sum, dtype=np.float32)

    sq = np.sqrt(w_sum).astype(np.float32)
    P0 = W[0] * sq[:, None, None]  # [M(k), M(m), C]
    P1 = W[1] * sq[:, None, None]
    S0 = 2.0 ** math.floor(math.log2(192.0 / float(P0.max())))
    S1 = 2.0 ** math.floor(math.log2(192.0 / float(P1.max())))
    Q0 = (P0 * np.float32(S0)).astype(fp8)
    Q1 = (P1 * np.float32(S1)).astype(fp8)

    in_maps = []
    for c in range(NCORES):
        k0 = c * KSH
        q0 = Q0[k0 : k0 + KSH].reshape(KM, C)
        q1 = Q1[k0 : k0 + KSH].reshape(KM, C)
        # x0 block: [p, j, h, i, m] = q0[j*256 + i*128 + p, h*128 + m]
        t0 = q0.reshape(NC2, 2, 128, 2, 128).transpose(2, 0, 3, 1, 4)
        t0 = t0.reshape(128, NC2, 512)
        # x1 block: [p, j, i, n] = q1[j*256 + i*128 + p, n]
        t1 = q1.reshape(NC2, 2, 128, C).transpose(2, 0, 1, 3)
        t1 = t1.reshape(128, NC2, 512)
        xwc = np.ascontiguousarray(
            np.concatenate([t0, t1], axis=2).reshape(128, NC2 * CW)
        )
        in_maps.append({"xw": xwc})
    return in_maps, S0, S1


def _run(in_maps, **kwargs):
    from concourse.bass_utils import run_bass_kernel_spmd

    if "nc" not in _cache:
        _cache["nc"] = _build_program()
    return run_bass_kernel_spmd(
        _cache["nc"], in_maps, core_ids=list(range(NCORES)), **kwargs
    )


def _finish(res, x, S0, S1):
    x = np.asarray(x)
    asum = np.zeros((2, 128, C), dtype=np.float64)
    for r in res.results:
        asum[0] += r["aout0"].astype(np.float64)
        asum[1] += r["aout1"].astype(np.float64)
    # A[c0, c1] with c0 = h*128 + p
    A = asum.reshape(256, 256)
    vals = A[x[:, 0].astype(np.int64), x[:, 1].astype(np.int64)]
    return (np.log(vals) - math.log(S0 * S1)).astype(np.float32)


def kernel(x, W, w_sum):
    in_maps, S0, S1 = _prep_inputs(x, W, w_sum)
    res = _run(in_maps)
    return _finish(res, x, S0, S1)


# revision 23
# speedup vs baseline: 1.1084x; 1.0428x over previous
"""HCLT probabilistic-circuit kernel for 8 Trainium2 NeuronCores.

Math: the reference collapses algebraically. With
  lp0 + lp1 summed in log space, exp'd, mixed by w_sum, then logsumexp'd,
the whole network is
  out[b] = log( sum_{k,m} w_sum[k] * W0[k,m,x0_b] * W1[k,m,x1_b] )
        = log( A[x0_b, x1_b] ),   A = sum_k w_k * W0[k].T @ W1[k]  (shape [C, C])

Distribution: shard the latent axis k (256) across 8 cores (32 each). Each core
reads its W shard quantized to fp8e4 (sqrt(w_sum) folded into both factors plus
power-of-two range scales) and accumulates the partial A_c via DoubleRow fp8
matmuls (256 contraction rows per instruction at 2x rate). The host sums the 8
partial A_c [256, 256] f32 outputs, gathers at the 1024 (x0_b, x1_b) positions,
removes the scales, and takes the log.

Layout: both W factors live in ONE dram tensor, interleaved per 256-row chunk
(x0-block 512B | x1-block 512B per partition row), so a single DMA trigger
(~0.8us each on the issuing engine) feeds both matmul operands. Pieces are
graduated (tiny first) so the PE starts as soon as possible, and alternate
between the two trigger engines/queues. The partial A is DMA'd straight out
of PSUM, split across both queues.
"""

import math
import sys

import numpy as np

sys.path.insert(0, "/opt/trn_rl_repo")

import ml_dtypes

B, V, M, C = 1024, 2, 256, 256
NCORES = 8
KSH = M // NCORES          # k per core = 32
KM = KSH * M               # flattened contraction rows per core = 8192
NC2 = KM // 256            # 32 DoubleRow chunks of 256 rows
CW = 1024                  # sbuf columns per chunk: [x0 512 | x1 512]
# pieces as (start_chunk, end_chunk, engine): engine 0 = sync/q1,
# 1 = scalar/q10.  Uniform 4-chunk pieces keep DMA packets at 4KB (packet
# width = piece row width; small packets tank throughput).  Each queue
# streams its own contiguous chunk range; MM emission alternates between
# the queues' piece streams (PSUM accumulation is order-independent, so
# chunk order is free).
# q1 (sync) starts ~1.5us earlier than q10 (scalar), so it carries more.
# every piece is split across BOTH queues (q1 takes the first s1 chunks,
# q10 the rest), so piece arrival = max(queue progress) is monotone by
# construction and MM emission order can never mismatch arrivals, whatever
# the per-queue rate jitter.  q1 (earlier start) carries 18/32 chunks.
# (start_chunk, end_chunk, q1_share)
PIECES = [
    (0, 4, 3),
    (4, 8, 2),
    (8, 12, 2),
    (12, 16, 2),
    (16, 20, 2),
    (20, 24, 2),
    (24, 28, 2),
    (28, 32, 3),
]

_cache = {}


def _build_program():
    import concourse.bacc as bacc
    import concourse.mybir as mybir
    from concourse.tile import TileContext

    bf16 = mybir.dt.bfloat16
    f32 = mybir.dt.float32
    fp8 = mybir.dt.float8e4

    nc = bacc.Bacc("TRN2", target_bir_lowering=False)

    # per-chunk free layout: x0: [h(2), i(2), m(128)] then x1: [i(2), n(256)]
    xw = nc.dram_tensor("xw", [128, NC2 * CW], fp8, kind="ExternalInput")
    aout0 = nc.dram_tensor("aout0", [128, C], bf16, kind="ExternalOutput")
    aout1 = nc.dram_tensor("aout1", [128, C], bf16, kind="ExternalOutput")

    with TileContext(nc) as tc:
        with (
            tc.tile_pool(name="wp", bufs=1) as wp,
            tc.tile_pool(name="apool", bufs=1, space="PSUM") as apool,
        ):
            xsb = wp.tile([128, NC2 * CW], fp8, name="xsb")

            # each piece split across both queues; triggers issue in
            # piece order on each engine
            for eng_id in (0, 1):
                eng = nc.sync if eng_id == 0 else nc.scalar
                for a, b, s1 in PIECES:
                    lo = a * CW if eng_id == 0 else (a + s1) * CW
                    hi = (a + s1) * CW if eng_id == 0 else b * CW
                    if lo < hi:
                        eng.dma_start(out=xsb[:, lo:hi], in_=xw[:, lo:hi])

            a_ps = []
            for h in range(2):
                ah = apool.tile([128, C], f32, name=f"a{h}")
                a_ps.append(ah)

            nmm = [0, 0]
            for a, b, _s1 in PIECES:
                for h in range(2):
                    for j in range(a, b):
                        lhsT = xsb[
                            :, j * CW + h * 256 : j * CW + (h + 1) * 256
                        ].rearrange("p (i m) -> p i m", i=2)
                        rhs = xsb[:, j * CW + 512 : (j + 1) * CW].rearrange(
                            "p (i n) -> p i n", i=2
                        )
                        nmm[h] += 1
                        nc.tensor.matmul(
                            a_ps[h],
                            lhsT=lhsT,
                            rhs=rhs,
                            start=(nmm[h] == 1),
                            stop=(nmm[h] == NC2),
                            perf_mode=mybir.MatmulPerfMode.DoubleRow,
                        )

            # PSUM -> SBUF (bf16) on two engines in parallel, then one
            # output DMA per queue
            asb0 = wp.tile([128, C], bf16, name="asb0")
            asb1 = wp.tile([128, C], bf16, name="asb1")
            nc.vector.tensor_copy(asb0, a_ps[0])
            nc.scalar.copy(asb1, a_ps[1])
            nc.sync.dma_start(out=aout0[:], in_=asb0[:])
            nc.scalar.dma_start(out=aout1[:], in_=asb1[:])

    nc.compile()
    return nc


def _prep_inputs(x, W, w_sum):
    fp8 = ml_dtypes.float8_e4m3
    x = np.asarray(x)
    W = np.asarray(W, dtype=np.float32)
    w_sum = np.asarray(w_sum, dtype=np.float32)

    sq = np.sqrt(w_sum).astype(np.float32)
    P0 = W[0] * sq[:, None, None]  # [M(k), M(m), C]
    P1 = W[1] * sq[:, None, None]
    S0 = 2.0 ** math.floor(math.log2(192.0 / float(P0.max())))
    S1 = 2.0 ** math.floor(math.log2(192.0 / float(P1.max())))
    Q0 = (P0 * np.float32(S0)).astype(fp8)
    Q1 = (P1 * np.float32(S1)).astype(fp8)

    in_maps = []
    for c in range(NCORES):
        k0 = c * KSH
        q0 = Q0[k0 : k0 + KSH].reshape(KM, C)
        q1 = Q1[k0 : k0 + KSH].reshape(KM, C)
        # x0 block: [p, j, h, i, m] = q0[j*256 + i*128 + p, h*128 + m]
        t0 = q0.reshape(NC2, 2, 128, 2, 128).transpose(2, 0, 3, 1, 4)
        t0 = t0.reshape(128, NC2, 512)
        # x1 block: [p, j, i, n] = q1[j*256 + i*128 + p, n]
        t1 = q1.reshape(NC2, 2, 128, C).transpose(2, 0, 1, 3)
        t1 = t1.reshape(128, NC2, 512)
        xwc = np.ascontiguousarray(
            np.concatenate([t0, t1], axis=2).reshape(128, NC2 * CW)
        )
        in_maps.append({"xw": xwc})
    return in_maps, S0, S1


def _run(in_maps, **kwargs):
    from concourse.bass_utils import run_bass_kernel_spmd

    if "nc" not in _cache:
        _cache["nc"] = _build_program()
    return run_bass_kernel_spmd(
        _cache["nc"], in_maps, core_ids=list(range(NCORES)), **kwargs
    )


def _finish(res, x, S0, S1):
    x = np.asarray(x)
    asum = np.zeros((2, 128, C), dtype=np.float64)
    for r in res.results:
        asum[0] += r["aout0"].astype(np.float64)
        asum[1] += r["aout1"].astype(np.float64)
    # A[c0, c1] with c0 = h*128 + p
    A = asum.reshape(256, 256)
    vals = A[x[:, 0].astype(np.int64), x[:, 1].astype(np.int64)]
    return (np.log(vals) - math.log(S0 * S1)).astype(np.float32)


def kernel(x, W, w_sum):
    in_maps, S0, S1 = _prep_inputs(x, W, w_sum)
    res = _run(in_maps)
    return _finish(res, x, S0, S1)
